# revision 41
# baseline (speedup 1.0000x reference)
"""MLA (multi-head latent attention) Trainium2 Bass kernel, 8-core SPMD.

Sharding: 2-way data parallel over batch x 4-way tensor parallel over heads.
Core c handles batch b = c // 4 and heads [hg*4, hg*4+4) with hg = c % 4.
Each core computes the full MLA forward for its batch/heads and the partial
out-projection (row-sharded W_o); the host sums the 4 partials per batch.

Fully fused per-chunk pipeline. For each 512-token chunk g:
  S1: q (folded W_dq@[W_uq|W_qr], direct from x), krx, ropes, c_kv chains
  S2: k up-proj, v up-proj
  O(g-1): previous chunk's out-projection (interleaved per head)
  A(g): causal attention for query chunk g over key chunks 0..g

Precision plan: the three projections that only influence attention logits
(q folded, k-rope from x, k-content up-proj) run as fp8e4 DoubleRow matmul
chains (2 k-tiles per instruction, 2 rows/cycle): weights are pre-scaled by
32 on the host, compensated by a 1/32 scale at PSUM eviction. x is shipped
both as fp8 (for those) and fp16 (for c_kv). c_kv/v/scores/AV/out-proj stay
fp16 (their quantization error would land directly in the output).

Engine balance: PE does matmuls only. Scalar does exp and ALL PSUM
evictions (fast PSUM reads). Vector does only fp16-SBUF 4x-mode work:
rope muls/add/sub, mask muls, single fp16 Z accumulator per head, the
1/Z normalize mul. GpSimd does half the rope row copies and the
partition-broadcast of 1/Z. Z per head: za (fp16 pair accum) -> zr
(pair sum) -> ones-matmul (partition sum, PSUM [1,512]) -> reciprocal ->
gpsimd broadcast to [128,512] -> one vector mul at attn eviction. Each
head's Z chain is deferred into the next head's score stream; the last
head's into the next chunk's S1. DMA queues: xt8 on scalar, xt16 on
vector, weights + output on sync, tables on gpsimd; chunk-0 x tiles are
split in half so the first matmul chain starts after ~1MB of DMA.

Layouts on device (partition dim first):
  xT16/xT8 [128, 8, S]x2 x[b].T halves, feature-on-partition
  qTg      [128, 4, 512] per head: rows 0:64 content, 64:96/96:128 rope
  kT       [128, 4, S]   same row layout
  v        [128, 16, 512] [token%128, token//128, head*128+d]
  scoresT  [128k, 2, 512q] PSUM pair; exp'd on Scalar -> et (fp16)
Out-projection PSUM evicts to fp16 and the output DRAM tensor is fp16
(host sums the 4 partials per batch in fp32).
"""
import sys

sys.path.insert(0, "/opt/trn_rl_repo")

import numpy as np
import ml_dtypes

import concourse.bacc as bacc
import concourse.mybir as mybir
import concourse.tile as tile
from concourse import bass_utils

H_DIM = 2048
N_HEADS = 16
D_HEAD = 128
D_ROPE = 64
D_NOPE = 64
HALF = D_ROPE // 2          # 32
C_DIM = 512
ROPE_BASE = 10000.0
HPC = 4                     # heads per core
B = 2
S_FULL = 2048
KC = H_DIM // 128           # 16
KCH = KC // 2               # 8 (one x half-tile)
CC = C_DIM // 128           # 4
SCALE = 1.0 / float(np.sqrt(D_HEAD))
WS = 32.0                   # fp8 weight pre-scale (host), undone at evict

FP8_Q = True
FP8_KR = True
FP8_KUP = True
USE_GPB = True              # gpsimd partition_broadcast for 1/Z

f8 = mybir.dt.float8e4
f16 = mybir.dt.float16
f32 = mybir.dt.float32
npf8 = ml_dtypes.float8_e4m3
DR = mybir.MatmulPerfMode.DoubleRow


def build_nc(S=S_FULL):
    TC = S // 512            # token chunks / query groups

    nc = bacc.Bacc("TRN2", target_bir_lowering=False, debug=False)

    d_xT8 = nc.dram_tensor("xT8", [TC, 128, KC * 512], f8, kind="ExternalInput")
    d_xT16 = nc.dram_tensor("xT16", [TC, 128, KC * 512], f16, kind="ExternalInput")
    qdt = f8 if FP8_Q else f16
    krdt = f8 if FP8_KR else f16
    kudt = f8 if FP8_KUP else f16
    d_wq = nc.dram_tensor("wq", [128, KC, 512], qdt, kind="ExternalInput")
    d_wkrx1 = nc.dram_tensor("wkrx1", [128, KC, HPC * HALF], krdt, kind="ExternalInput")
    d_wkrx2 = nc.dram_tensor("wkrx2", [128, KC, HPC * HALF], krdt, kind="ExternalInput")
    d_wdkv = nc.dram_tensor("wdkv", [128, KC, C_DIM], f16, kind="ExternalInput")
    d_wuk = nc.dram_tensor("wuk", [128, CC, HPC * D_NOPE], kudt, kind="ExternalInput")
    d_wuv = nc.dram_tensor("wuv", [128, CC, HPC * D_HEAD], f16, kind="ExternalInput")
    d_wo = nc.dram_tensor("wo", [128, HPC, H_DIM], f16, kind="ExternalInput")
    d_cos = nc.dram_tensor("cosA", [128, S], f16, kind="ExternalInput")
    d_sin = nc.dram_tensor("sinA", [128, S], f16, kind="ExternalInput")
    d_mask = nc.dram_tensor("masks", [128, 4, 512], f16, kind="ExternalInput")
    d_onec = nc.dram_tensor("onec", [128, 1], f16, kind="ExternalInput")
    d_oner = nc.dram_tensor("oner", [1, 128], f16, kind="ExternalInput")
    d_o = nc.dram_tensor("o", [S, H_DIM], f16, kind="ExternalOutput")

    import contextlib
    with tile.TileContext(nc) as tc:
        with contextlib.ExitStack() as stack:
            def pool(name, **kw):
                return stack.enter_context(tc.tile_pool(name=name, **kw))

            p_const = pool("const", bufs=1)
            p_w = pool("w", bufs=1, side="right")
            p_x8 = pool("x8", bufs=2, side="right")
            p_x16 = pool("x16", bufs=2, side="right")
            p_kT = pool("kT", bufs=1)
            p_v = pool("vp", bufs=1)
            p_qT = pool("qT", bufs=1)
            p_ckv = pool("ckv", bufs=1, side="right")
            p_ckv8 = pool("ckv8", bufs=1, side="right")
            p_rx = pool("rx", bufs=1, side="right")
            p_tmp = pool("tmp", bufs=2, side="right")
            p_et = pool("et", bufs=5)
            p_za = pool("za", bufs=2)
            p_zr = pool("zr", bufs=2)
            p_rz = pool("rz", bufs=2)
            p_att = pool("att", bufs=2)
            p_ot = pool("ot", bufs=2)
            p_psA = pool("psA", bufs=2, space="PSUM")
            p_po = pool("ps_po", bufs=2, space="PSUM")
            p_pz = pool("ps_z", bufs=1, space="PSUM")
            if not USE_GPB:
                p_pb = pool("ps_b", bufs=1, space="PSUM")

            # ---- constants + weights, ordered by first use.
            # sync queue: weights (and later the output tiles).
            # scalar queue: xt8 chunks; vector queue: xt16 chunks.
            # gpsimd queue: tables/masks/ones.
            wq = p_w.tile([128, KC, 512], qdt, tag="wq")
            for k0 in range(0, KC, 4):
                nc.sync.dma_start(wq[:, k0:k0 + 4, :],
                                  d_wq.ap()[:, k0:k0 + 4, :])
            wkrx1 = p_w.tile([128, KC, HPC * HALF], krdt, tag="wkrx1")
            nc.gpsimd.dma_start(wkrx1[:], d_wkrx1.ap())
            wkrx2 = p_w.tile([128, KC, HPC * HALF], krdt, tag="wkrx2")
            nc.gpsimd.dma_start(wkrx2[:], d_wkrx2.ap())
            wdkv = p_w.tile([128, KC, C_DIM], f16, tag="wdkv")
            nc.sync.dma_start(wdkv[:, 0:KCH, :], d_wdkv.ap()[:, 0:KCH, :])
            nc.sync.dma_start(wdkv[:, KCH:KC, :], d_wdkv.ap()[:, KCH:KC, :])
            wuk = p_w.tile([128, CC, HPC * D_NOPE], kudt, tag="wuk")
            nc.sync.dma_start(wuk[:], d_wuk.ap())
            wuv = p_w.tile([128, CC, HPC * D_HEAD], f16, tag="wuv")
            nc.sync.dma_start(wuv[:], d_wuv.ap())
            wo = p_w.tile([128, HPC, H_DIM], f16, tag="wo")
            nc.sync.dma_start(wo[:], d_wo.ap())
            cosA = p_const.tile([128, S], f16, tag="cosA")
            nc.gpsimd.dma_start(cosA[:], d_cos.ap())
            sinA = p_const.tile([128, S], f16, tag="sinA")
            nc.gpsimd.dma_start(sinA[:], d_sin.ap())

            def dma_x(g):
                # xt8 on the scalar queue, xt16 on the gpsimd queue; issued
                # one chunk ahead (before the previous chunk's attention) so
                # they are not stuck behind that chunk's exp stream
                xt8a = p_x8.tile([128, KCH, 512], f8, tag="x8a")
                if g == 0:
                    h = KCH // 2
                    nc.scalar.dma_start(xt8a[:, 0:h, :],
                                        d_xT8.ap()[g][:, 0:h * 512])
                    nc.scalar.dma_start(xt8a[:, h:KCH, :],
                                        d_xT8.ap()[g][:, h * 512:KCH * 512])
                else:
                    nc.scalar.dma_start(xt8a[:], d_xT8.ap()[g][:, 0:KCH * 512])
                xt8b = p_x8.tile([128, KCH, 512], f8, tag="x8b")
                nc.scalar.dma_start(xt8b[:],
                                    d_xT8.ap()[g][:, KCH * 512:KC * 512])
                xt16a = p_x16.tile([128, KCH, 512], f16, tag="x16a")
                nc.scalar.dma_start(xt16a[:], d_xT16.ap()[g][:, 0:KCH * 512])
                xt16b = p_x16.tile([128, KCH, 512], f16, tag="x16b")
                nc.scalar.dma_start(xt16b[:],
                                    d_xT16.ap()[g][:, KCH * 512:KC * 512])
                return xt8a, xt8b, xt16a, xt16b

            xtiles = dma_x(0)
            masks = p_const.tile([128, 4, 512], f16, tag="masks")
            nc.gpsimd.dma_start(masks[:], d_mask.ap())
            onech = p_const.tile([128, 1], f16, tag="onec")
            nc.gpsimd.dma_start(onech[:], d_onec.ap())
            oner = p_const.tile([1, 128], f16, tag="oner")
            nc.gpsimd.dma_start(oner[:], d_oner.ap())

            # persistent K/V for all chunks
            kT = p_kT.tile([128, HPC, S], f16, tag="kT")
            v_sb = p_v.tile([128, S // 128, 512], f16, tag="v")

            def dr_chain(dst_ap, w_sb, c0, cn, xa, xb, fp8_on):
                """Contraction chain over all KC k-tiles of x into dst_ap.

                fp8 path: DoubleRow pairs; fp16 path: plain chain."""
                if fp8_on:
                    for k2 in range(KCH // 2):
                        nc.tensor.matmul(
                            dst_ap, w_sb[:, 2 * k2:2 * k2 + 2, c0:c0 + cn],
                            xa[:, 2 * k2:2 * k2 + 2, :],
                            start=(k2 == 0), stop=False, perf_mode=DR)
                    for k2 in range(KCH // 2):
                        nc.tensor.matmul(
                            dst_ap,
                            w_sb[:, KCH + 2 * k2:KCH + 2 * k2 + 2, c0:c0 + cn],
                            xb[:, 2 * k2:2 * k2 + 2, :],
                            start=False, stop=(k2 == KCH // 2 - 1),
                            perf_mode=DR)
                else:
                    for k in range(KC):
                        src = xa if k < KCH else xb
                        nc.tensor.matmul(
                            dst_ap, w_sb[:, k, c0:c0 + cn],
                            src[:, k % KCH, :],
                            start=(k == 0), stop=(k == KC - 1))

            def rope(xx, dst, gs, dc):
                # xx: [128 = 4h*32, 2, 512] fp16 SBUF (j0 = x1, j1 = x2)
                t1 = p_tmp.tile([128, 512], f16, tag="t1")
                t2 = p_tmp.tile([128, 512], f16, tag="t2")
                o12 = p_tmp.tile([128, 2, 512], f16, tag="o12")
                nc.vector.tensor_mul(t1[:], xx[:, 0, :], cosA[:, gs])
                nc.vector.tensor_mul(t2[:], xx[:, 1, :], sinA[:, gs])
                nc.vector.tensor_sub(o12[:, 0, :], t1[:], t2[:])
                t3 = p_tmp.tile([128, 512], f16, tag="t3")
                t4 = p_tmp.tile([128, 512], f16, tag="t4")
                nc.vector.tensor_mul(t3[:], xx[:, 0, :], sinA[:, gs])
                nc.vector.tensor_mul(t4[:], xx[:, 1, :], cosA[:, gs])
                nc.vector.tensor_add(o12[:, 1, :], t3[:], t4[:])
                for h in range(HPC):
                    hs = slice(h * HALF, (h + 1) * HALF)
                    e1 = nc.scalar if h % 2 == 0 else nc.gpsimd
                    e2 = nc.gpsimd if h % 2 == 0 else nc.scalar
                    (e1.copy if e1 is nc.scalar else e1.tensor_copy)(
                        dst[64:96, h, dc], o12[hs, 0, :])
                    (e2.copy if e2 is nc.scalar else e2.tensor_copy)(
                        dst[96:128, h, dc], o12[hs, 1, :])

            def z1a(za):
                # pair-sum of the Z accumulator; issued early (vector queue
                # is shallow at head start)
                zr = p_zr.tile([128, 512], f16, tag="zr")
                nc.vector.tensor_add(zr[:], za[:, 0, :], za[:, 1, :])
                return zr

            def z1b(zr):
                pz = p_pz.tile([1, 512], f32, tag="pz")
                nc.tensor.matmul(pz[:], onech[:], zr[:], start=True, stop=True)
                r0 = p_rz.tile([1, 512], f32, tag="r0")
                nc.vector.reciprocal_approx_fast(r0[:], pz[:])
                rzc = p_rz.tile([1, 512], f16, tag="rzc")
                nc.vector.tensor_copy(rzc[:], r0[:])
                rb = p_zr.tile([128, 512], f16, tag="rb")
                if USE_GPB:
                    nc.gpsimd.partition_broadcast(rb[:], rzc[:])
                else:
                    pb = p_pb.tile([128, 512], f32, tag="pb")
                    nc.tensor.matmul(pb[:], oner[:], rzc[:], start=True,
                                     stop=True)
                    nc.scalar.copy(rb[:], pb[:])
                return rb

            def z2(h, rb, po, attn_t):
                araw = p_zr.tile([128, 512], f16, tag="araw")
                nc.scalar.copy(araw[:], po[:])
                nc.vector.tensor_mul(attn_t[:, h, :], araw[:], rb[:])

            def emit_O_t4(og, attn_t, t4):
                # out-projection for token subtile t4 of chunk og
                tt = og * 4 + t4
                for np2 in range(2):
                    pso = p_psA.tile([128, 2, 512], f32, tag="ps")
                    for j in range(2):
                        nck = 2 * np2 + j
                        for h in range(HPC):
                            nc.tensor.matmul(
                                pso[:, j, :],
                                attn_t[:, h, t4 * 128:(t4 + 1) * 128],
                                wo[:, h, nck * 512:(nck + 1) * 512],
                                start=(h == 0), stop=(h == HPC - 1),
                            )
                    ot = p_ot.tile([128, 2, 512], f16, tag="ot")
                    if (t4 + np2) % 2 == 0:
                        nc.scalar.copy(ot[:], pso[:])
                    else:
                        nc.vector.tensor_copy(ot[:], pso[:])
                    nc.sync.dma_start(
                        d_o.ap()[tt * 128:(tt + 1) * 128,
                                 np2 * 1024:(np2 + 1) * 1024],
                        ot[:])

            carryZ = None   # last head of prev chunk: (h, za, po, attn)
            carryO = None   # previous chunk's out-projection: (g, attn)

            for g in range(TC):
                gs = slice(g * 512, (g + 1) * 512)
                xt8a, xt8b, xt16a, xt16b = xtiles
                xqa, xqb = (xt8a, xt8b) if FP8_Q else (xt16a, xt16b)
                xra, xrb = (xt8a, xt8b) if FP8_KR else (xt16a, xt16b)
                qsc = 1.0 / WS if FP8_Q else 1.0
                rsc = 1.0 / WS if FP8_KR else 1.0
                usc = 1.0 / WS if FP8_KUP else 1.0

                # ---- S1a: q direct from x (folded weights) ----
                qTg = p_qT.tile([128, HPC, 512], f16, tag="qTg")
                qx = p_rx.tile([128, 2, 512], f16, tag="qx")
                for mp in range(2):
                    ps = p_psA.tile([128, 2, 512], f32, tag="ps")
                    for j in range(2):
                        m = 2 * mp + j
                        dr_chain(ps[:, j, :], wq, m * 128, 128, xqa, xqb,
                                 FP8_Q)
                    if mp == 0:
                        for pp in range(2):
                            # heads {0,2} from psum rows 0:64, {1,3} from 64:128
                            nc.scalar.activation(
                                qTg[0:64, pp:pp + 3:2, :],
                                ps[64 * pp:64 * pp + 64, 0:2, :],
                                mybir.ActivationFunctionType.Copy, scale=qsc)
                        if carryZ is not None:
                            ch, czr, cpo, cattn = carryZ
                            crb = z1b(czr)
                            carryZ = (ch, crb, cpo, cattn)
                    else:
                        nc.scalar.activation(
                            qx[:], ps[:],
                            mybir.ActivationFunctionType.Copy, scale=qsc)
                        if carryZ is not None:
                            ch, crb, cpo, cattn = carryZ
                            z2(ch, crb, cpo, cattn)
                            carryZ = None

                # ---- S1b: krx chains ----
                kx = p_rx.tile([128, 2, 512], f16, tag="kx")
                ps = p_psA.tile([128, 2, 512], f32, tag="ps")
                for j, w_sb in enumerate((wkrx1, wkrx2)):
                    dr_chain(ps[:, j, :], w_sb, 0, HPC * HALF, xra, xrb,
                             FP8_KR)
                nc.scalar.activation(kx[:], ps[:],
                                     mybir.ActivationFunctionType.Copy,
                                     scale=rsc)

                # ---- ropes (vector, fp16 SBUF) ----
                rope(qx, qTg, gs, slice(0, 512))
                rope(kx, kT, gs, gs)

                # ---- S1c: c_kv chains (fp16) + dual evict ----
                ckvg = p_ckv.tile([128, CC, 512], f16, tag="ckvg")
                ckv8 = None
                if FP8_KUP:
                    ckv8 = p_ckv8.tile([128, CC, 512], f8, tag="ckv8")
                for mp in range(2):
                    ps = p_psA.tile([128, 2, 512], f32, tag="ps")
                    for j in range(2):
                        m = 2 * mp + j
                        dr_chain(ps[:, j, :], wdkv, m * 128, 128,
                                 xt16a, xt16b, False)
                    # fp8 copy first: the k-up chain consumes it soonest
                    if FP8_KUP:
                        nc.scalar.copy(ckv8[:, 2 * mp:2 * mp + 2, :], ps[:])
                    if mp == 0:
                        nc.scalar.copy(ckvg[:, 0:2, :], ps[:])
                    else:
                        nc.vector.tensor_copy(ckvg[:, 2:4, :], ps[:])

                # previous chunk's first out-proj group here: it gives the
                # PE ~3.4us of independent work that covers the mp1 ckv
                # evictions, so the k-up chain doesn't stall on scalar
                if carryO is not None:
                    emit_O_t4(carryO[0], carryO[1], 0)

                # ---- S2a: k up-proj (content) ----
                ps = p_psA.tile([128, 2, 512], f32, tag="ps")
                for m2 in range(2):
                    if FP8_KUP:
                        for k2 in range(CC // 2):
                            nc.tensor.matmul(
                                ps[:, m2, :],
                                wuk[:, 2 * k2:2 * k2 + 2,
                                    m2 * 128:(m2 + 1) * 128],
                                ckv8[:, 2 * k2:2 * k2 + 2, :],
                                start=(k2 == 0), stop=(k2 == CC // 2 - 1),
                                perf_mode=DR)
                    else:
                        for k in range(CC):
                            nc.tensor.matmul(
                                ps[:, m2, :],
                                wuk[:, k, m2 * 128:(m2 + 1) * 128],
                                ckvg[:, k, :],
                                start=(k == 0), stop=(k == CC - 1))
                for pp in range(2):
                    nc.scalar.activation(
                        kT[0:64, pp:pp + 3:2, gs],
                        ps[64 * pp:64 * pp + 64, 0:2, :],
                        mybir.ActivationFunctionType.Copy, scale=usc)

                # ---- S2b: v up-proj (fp16) ----
                for tp in range(2):
                    ps = p_psA.tile([128, 2, 512], f32, tag="ps")
                    for j in range(2):
                        tt = 2 * tp + j
                        for k in range(CC):
                            nc.tensor.matmul(
                                ps[:, j, :],
                                ckvg[:, k, tt * 128:(tt + 1) * 128],
                                wuv[:, k, :], start=(k == 0),
                                stop=(k == CC - 1))
                    nc.scalar.copy(
                        v_sb[:, g * 4 + 2 * tp:g * 4 + 2 * tp + 2, :], ps[:])

                # prefetch next chunk's x tiles ahead of the exp stream
                if g + 1 < TC:
                    xtiles = dma_x(g + 1)

                # ---- A(g): attention; previous chunk's out-projection is
                # interleaved one t4-group per head ----
                attn_g = p_att.tile([128, HPC, 512], f16, tag="attn")
                nkt = 4 * (g + 1)
                nktp = nkt // 2
                LOOKP = 3
                ktp_order = ([(4 * g, 0), (4 * g + 2, 2)]
                             + [(2 * i, -1) for i in range(2 * g)])
                zchain = None
                zs1 = min(4, nktp)
                zs2 = min(6, nktp + LOOKP - 1)
                for h in range(HPC):
                    po = p_po.tile([128, 512], f32, tag="po")
                    za = None
                    et0 = None
                    ets = {}
                    for step in range(nktp + LOOKP):
                        if step < nktp:
                            kt0, d = ktp_order[step]
                            ps = p_psA.tile([128, 2, 512], f32, tag="ps")
                            for j in range(2):
                                # causal trim: diagonal tile dd only sees
                                # queries q >= dd*128
                                dd = (kt0 + j - 4 * g) if d >= 0 else 0
                                vs = slice(dd * 128, 512)
                                nc.tensor.matmul(
                                    ps[:, j, vs],
                                    kT[:, h,
                                       (kt0 + j) * 128:(kt0 + j + 1) * 128],
                                    qTg[:, h, vs], start=True, stop=True)
                            et = p_et.tile([128, 2, 512], f16, tag="et")
                            nc.scalar.activation(
                                et[:], ps[:],
                                mybir.ActivationFunctionType.Exp, scale=SCALE)
                            if d >= 0:
                                nc.vector.tensor_mul(et[:], et[:],
                                                     masks[:, d:d + 2, :])
                                # zero the dead (never-computed) q regions so
                                # the full-width za add and AV stay correct
                                for j in range(2):
                                    dd = kt0 + j - 4 * g
                                    if dd > 0:
                                        nc.vector.memset(
                                            et[:, j, 0:dd * 128], 0)
                            if step == 0:
                                et0 = et
                            elif step == 1:
                                za = p_za.tile([128, 2, 512], f16, tag="za")
                                nc.vector.tensor_add(za[:], et0[:], et[:])
                            else:
                                nc.vector.tensor_add(za[:], za[:], et[:])
                            ets[step] = et
                        if step == zs1 and zchain is not None:
                            ph, pzr, ppo = zchain
                            prb = z1b(pzr)
                            zchain = (ph, prb, ppo, True)
                        if step == zs2 and zchain is not None:
                            ph, prb, ppo, _ = zchain
                            z2(ph, prb, ppo, attn_g)
                            zchain = None
                        if (g == TC - 1 and h == HPC - 1 and step == nktp):
                            # final chunk, final head: resolve Z early so the
                            # epilogue out-projection starts sooner
                            carryZ = (h, z1b(z1a(za)), po, attn_g)
                        if step >= LOOKP:
                            kt0, d = ktp_order[step - LOOKP]
                            et = ets.pop(step - LOOKP)
                            for j in range(2):
                                first = (step == LOOKP and j == 0)
                                last = (step == nktp + LOOKP - 1 and j == 1)
                                dd = (kt0 + j - 4 * g) if d >= 0 else 0
                                if first or last:
                                    dd = 0  # start/stop ops keep full width
                                vs = slice(dd * 128, 512)
                                nc.tensor.matmul(
                                    po[:, vs],
                                    v_sb[:, kt0 + j, h * 128:(h + 1) * 128],
                                    et[:, j, vs],
                                    start=first, stop=last)
                    if h < HPC - 1:
                        zchain = (h, z1a(za), po)
                    elif g < TC - 1:
                        carryZ = (h, z1a(za), po, attn_g)
                    if carryO is not None and h < HPC - 1:
                        emit_O_t4(carryO[0], carryO[1], h + 1)
                carryO = (g, attn_g)

            # ---- epilogue: flush the last chunk's Z chain + out-projection
            ch, crb, cpo, cattn = carryZ
            z2(ch, crb, cpo, cattn)
            cog, cattn2 = carryO
            for t4 in range(4):
                emit_O_t4(cog, cattn2, t4)

    nc.compile()
    return nc


# ================= host-side prep =================

def _rope_tables(S):
    inv_freq = 1.0 / (ROPE_BASE ** (np.arange(HALF, dtype=np.float64) / HALF))
    ang = np.arange(S, dtype=np.float64)[:, None] * inv_freq[None, :]  # [S, 32]
    cosA = np.tile(np.cos(ang).T, (4, 1)).astype(np.float16)          # [128, S]
    sinA = np.tile(np.sin(ang).T, (4, 1)).astype(np.float16)
    return cosA, sinA


def _masks01():
    p = np.arange(128)[:, None]
    j = np.arange(512)[None, :]
    m = np.zeros((128, 4, 512), dtype=np.float16)
    for d in range(4):
        m[:, d, :] = (d * 128 + p <= j).astype(np.float16)
    return m


def _to8(w):
    return np.clip(w * WS, -240.0, 240.0).astype(npf8)


def _core_inputs(core, x, W_dq, W_dkv, W_uq, W_uk, W_uv, W_kr, W_qr, W_o, S):
    b, hg = core // 4, core % 4
    h0 = hg * HPC

    def pm(w):  # [R, C] -> [128, R//128, C] partition-major, keep dtype
        R, Cc = w.shape
        return np.ascontiguousarray(
            w.reshape(R // 128, 128, Cc).transpose(1, 0, 2))

    heads = np.arange(h0, h0 + HPC)
    rope_x1 = (heads[:, None] * D_ROPE + np.arange(HALF)[None, :]).reshape(-1)
    rope_x2 = rope_x1 + HALF
    nope_cols = (heads[:, None] * D_NOPE + np.arange(D_NOPE)[None, :]).reshape(-1)
    v_cols = (heads[:, None] * D_HEAD + np.arange(D_HEAD)[None, :]).reshape(-1)

    # fold W_dq @ [W_uq | W_qr] -> direct q weights [2048, 512]
    wq_cols = np.concatenate(
        [W_uq[:, nope_cols], W_qr[:, rope_x1], W_qr[:, rope_x2]], axis=1)
    wq = (W_dq.astype(np.float64) @ wq_cols.astype(np.float64)).astype(np.float32)

    xT = np.ascontiguousarray(x[b].T)                     # [2048, S]
    TCn = S // 512
    cosA, sinA = _rope_tables(S)
    xTp = pm(xT).reshape(128, KC, TCn, 512).transpose(2, 0, 1, 3)
    xTc = np.ascontiguousarray(xTp).reshape(TCn, 128, KC * 512)
    return {
        "xT8": np.clip(xTc, -240, 240).astype(npf8),
        "xT16": xTc.astype(np.float16),
        "wq": pm(_to8(wq) if FP8_Q else wq.astype(np.float16)),
        "wkrx1": pm(_to8(W_kr[:, rope_x1]) if FP8_KR
                    else W_kr[:, rope_x1].astype(np.float16)),
        "wkrx2": pm(_to8(W_kr[:, rope_x2]) if FP8_KR
                    else W_kr[:, rope_x2].astype(np.float16)),
        "wdkv": pm(W_dkv.astype(np.float16)),
        "wuk": pm(_to8(W_uk[:, nope_cols]) if FP8_KUP
                  else W_uk[:, nope_cols].astype(np.float16)),
        "wuv": pm(W_uv[:, v_cols].astype(np.float16)),
        "wo": pm(W_o[h0 * D_HEAD:(h0 + HPC) * D_HEAD, :].astype(np.float16)),
        "cosA": cosA,
        "sinA": sinA,
        "masks": _masks01(),
        "onec": np.ones((128, 1), np.float16),
        "oner": np.ones((1, 128), np.float16),
    }


_NC_CACHE = {}


def _get_nc(S):
    if S not in _NC_CACHE:
        _NC_CACHE[S] = build_nc(S)
    return _NC_CACHE[S]


def make_in_maps(inputs, S):
    args = (np.asarray(inputs["x"], np.float32),
            np.asarray(inputs["W_dq"], np.float32),
            np.asarray(inputs["W_dkv"], np.float32),
            np.asarray(inputs["W_uq"], np.float32),
            np.asarray(inputs["W_uk"], np.float32),
            np.asarray(inputs["W_uv"], np.float32),
            np.asarray(inputs["W_kr"], np.float32),
            np.asarray(inputs["W_qr"], np.float32),
            np.asarray(inputs["W_o"], np.float32))
    x, W_dq, W_dkv, W_uq, W_uk, W_uv, W_kr, W_qr, W_o = args
    return [
        _core_inputs(c, x, W_dq, W_dkv, W_uq, W_uk, W_uv, W_kr, W_qr, W_o, S)
        for c in range(8)
    ]


def kernel(x, W_dkv, W_dq, W_uq, W_uk, W_uv, W_kr, W_qr, W_o, _trace=False):
    S = x.shape[1]
    nc = _get_nc(S)
    in_maps = make_in_maps(dict(x=x, W_dq=W_dq, W_dkv=W_dkv, W_uq=W_uq,
                                W_uk=W_uk, W_uv=W_uv, W_kr=W_kr, W_qr=W_qr,
                                W_o=W_o), S)
    res = bass_utils.run_bass_kernel_spmd(nc, in_maps, core_ids=list(range(8)),
                                          trace=_trace)
    out = np.zeros((B, S, H_DIM), np.float32)
    for c in range(8):
        out[c // 4] += res.results[c]["o"].astype(np.float32)
    if _trace:
        kernel.last_exec_time_ns = res.exec_time_ns
        kernel.last_results = res
    return out
